# revision 2
# baseline (speedup 1.0000x reference)
"""GNN message-passing kernel for Trainium2 (8 NeuronCores, SPMD) — v4.

Computes: out = segment_sum(x[edge_index[0]], edge_index[1], num_segments=N)
  i.e. for each edge e: out[dst[e]] += x[src[e]]

Strategy:
  - Destination nodes are assigned to (core, group, pos) slots by a host-side
    balancer: 8 cores x 100 groups x 128 positions. Greedy balancing
    equalizes per-group edge counts so the core-uniform tile count carries
    only ~2-3% padding.
  - Source rows are 4-colored into chunks of <=32768 rows (int16 gather
    index range), greedily balancing each (core, group)'s edges across
    chunks; x rows are permuted accordingly and stored bf16 padded to 128
    cols (256B row pitch).
  - Gather: the Q7 dma_gather ucode, invoked directly with elem_size=64 bf16
    (128B payload) and elem_step=128 (256B row stride). The 256B-elem assert
    in bass.dma_gather is a transpose-only restriction; the non-transpose
    ucode handles any payload. One call per (bigwin, chunk); padding slots
    gather chunk-local row 0 and are cancelled by a -1 one-hot key, so no
    suffix trimming or count registers are needed.
  - Scatter per tile (128 edge slots -> 128 dst positions of one group):
    DVE builds a bf16 one-hot via tensor_scalar is_equal (4x_2p mode);
    PE matmul lhsT=onehot, rhs=msg accumulates into a node-major PSUM slab
    [128 pos, 64 feat] (free dim 64, bf16: ~27ns/tile).
  - 4 groups share a PSUM tile [128, 256] per bigwin; the Activation engine
    flushes PSUM -> SBUF [128, 6400] f32; one final DMA writes y; the host
    unpermutes rows.

The Bass program is identical across cores (SPMD): tile counts use the max
over cores; per-core data streams (idx16, dstl) differ.
"""
import numpy as np

N_NODES = 100000
D_FEAT = 64
N_CORES = 8
N_GRP = 100          # groups (128-dst-node windows) per core
SW = 128             # positions per group / onehot width
GRP_PER_BW = 4       # groups per PSUM bank (bigwin)
N_BW = N_GRP // GRP_PER_BW   # 25 bigwins per core
N_CK = 4             # source chunks (int16 index range)
CK_CAP = 32768       # max rows per chunk
P = 128              # edge slots per tile
XPAD = 128           # x row pitch in bf16 elements (256B)


def cdiv(a, b):
    return -(-a // b)


def _balance_nodes(dst):
    """Assign each node to (core, group, pos), balancing edge counts."""
    import heapq
    deg = np.bincount(dst, minlength=N_NODES).astype(np.int64)
    order = np.argsort(-deg, kind="stable")

    core_id = np.empty(N_NODES, np.int32)
    grp_id = np.empty(N_NODES, np.int32)
    pos_id = np.empty(N_NODES, np.int32)

    cap_core = N_GRP * SW
    heap = [(0, 0, c) for c in range(N_CORES)]
    heapq.heapify(heap)
    for n in order:
        while True:
            load, fill, c = heapq.heappop(heap)
            if fill < cap_core:
                break
        core_id[n] = c
        heapq.heappush(heap, (load + deg[n], fill + 1, c))

    for c in range(N_CORES):
        nodes = order[core_id[order] == c]
        gheap = [(0, 0, g) for g in range(N_GRP)]
        heapq.heapify(gheap)
        for n in nodes:
            while True:
                load, fill, g = heapq.heappop(gheap)
                if fill < SW:
                    break
            grp_id[n] = g
            pos_id[n] = fill
            heapq.heappush(gheap, (load + deg[n], fill + 1, g))
    return core_id, grp_id, pos_id


def _color_sources(src, gidx):
    """4-color source rows (chunk assignment) balancing each (core,group)'s
    edges across chunks. Returns color[N] int32."""
    E = src.shape[0]
    order_s = np.argsort(src, kind="stable")
    src_s = src[order_s]
    g_s = gidx[order_s]
    starts = np.searchsorted(src_s, np.arange(N_NODES + 1))

    deg = starts[1:] - starts[:-1]
    row_order = np.argsort(-deg, kind="stable")

    n_groups = N_CORES * N_GRP
    cnt = np.zeros((n_groups, N_CK), np.int32)
    fill = np.zeros(N_CK, np.int64)
    color = np.empty(N_NODES, np.int32)
    for r in row_order:
        gs = g_s[starts[r]:starts[r + 1]]
        if len(gs):
            scores = cnt[gs, :].sum(axis=0)
        else:
            scores = fill // 1024
        scores = scores + np.where(fill >= CK_CAP, 1 << 30, 0)
        ck = int(np.argmin(scores))
        color[r] = ck
        fill[ck] += 1
        if len(gs):
            np.add.at(cnt, (gs, ck), 1)
    return color


def _preprocess(edge_index):
    src = np.asarray(edge_index[0], dtype=np.int64)
    dst = np.asarray(edge_index[1], dtype=np.int64)
    E = src.shape[0]

    core_id, grp_id, pos_id = _balance_nodes(dst)
    gidx = (core_id[dst] * N_GRP + grp_id[dst]).astype(np.int64)
    color = _color_sources(src, gidx)

    # permute x rows: chunk-major
    new_order = np.argsort(color, kind="stable")       # orig row at new pos
    chunk_sizes = np.bincount(color, minlength=N_CK)
    chunk_base = np.zeros(N_CK + 1, np.int64)
    np.cumsum(chunk_sizes, out=chunk_base[1:])
    newpos = np.empty(N_NODES, np.int64)
    newpos[new_order] = np.arange(N_NODES)
    local_idx = newpos - chunk_base[color]             # per orig row
    assert local_idx.min() >= 0 and local_idx.max() < CK_CAP

    e_core = core_id[dst].astype(np.int64)
    e_bw = grp_id[dst].astype(np.int64) // GRP_PER_BW
    e_gi = grp_id[dst].astype(np.int64) % GRP_PER_BW
    e_ck = color[src].astype(np.int64)

    # run key: (core, bw, ck, gi)
    key = ((e_core * N_BW + e_bw) * N_CK + e_ck) * GRP_PER_BW + e_gi
    order = np.argsort(key, kind="stable")
    key_s = key[order]
    n_runs = N_CORES * N_BW * N_CK * GRP_PER_BW
    counts = np.bincount(key_s, minlength=n_runs).reshape(
        N_CORES, N_BW, N_CK, GRP_PER_BW)
    T_run = cdiv(counts.max(axis=0), P)        # [N_BW, N_CK, GRP_PER_BW]

    # tile offsets: bigwin-major, then (ck, gi)
    run_tile_off = np.zeros((N_BW, N_CK, GRP_PER_BW), np.int64)
    acc = 0
    for b in range(N_BW):
        for c in range(N_CK):
            for gi in range(GRP_PER_BW):
                run_tile_off[b, c, gi] = acc
                acc += T_run[b, c, gi]
    tot_tiles = int(acc)
    tot_slots = tot_tiles * P
    call_tiles = T_run.sum(axis=2)             # [N_BW, N_CK]
    bw_tiles = call_tiles.sum(axis=1)          # [N_BW]

    # per-edge slot assignment
    run_start = np.zeros(n_runs + 1, np.int64)
    np.cumsum(counts.reshape(-1), out=run_start[1:])
    rank = np.arange(E, dtype=np.int64) - run_start[key_s]
    b_e = (key_s // (N_CK * GRP_PER_BW)) % N_BW
    c_e = (key_s // GRP_PER_BW) % N_CK
    g_e = key_s % GRP_PER_BW
    tile_e = run_tile_off[b_e, c_e, g_e] + rank // P
    part_e = rank % P
    core_e = key_s // (N_BW * N_CK * GRP_PER_BW)

    lsrc_s = local_idx[src[order]]
    pos_s = pos_id[dst[order]].astype(np.int64)

    idx_cores, dstl_cores = [], []
    for cr in range(N_CORES):
        m = core_e == cr
        stream = np.zeros(tot_slots, np.int16)        # pad -> chunk row 0
        stream[tile_e[m] * P + part_e[m]] = lsrc_s[m].astype(np.int16)
        wrapped = stream.reshape(tot_slots // 16, 16).T
        idx_cores.append(np.tile(wrapped, (8, 1)).astype(np.int16))

        dstl = np.full((P, tot_tiles), -1.0, np.float32)
        dstl[part_e[m], tile_e[m]] = pos_s[m].astype(np.float32)
        dstl_cores.append(dstl)

    sched = dict(T_run=T_run, run_tile_off=run_tile_off, tot_tiles=tot_tiles,
                 tot_slots=tot_slots, call_tiles=call_tiles, bw_tiles=bw_tiles,
                 chunk_base=chunk_base)
    meta = dict(core_id=core_id, grp_id=grp_id, pos_id=pos_id,
                new_order=new_order)
    return sched, idx_cores, dstl_cores, meta


def _dma_gather_small(gp, mybir, out_ap, in_ap, idxs_ap, num_idxs, elem_size,
                      elem_step, queue_num):
    """dma_gather with sub-256B payload: elem_size elements per index from
    rows pitched elem_step elements apart (pitch must be 256B-divisible)."""
    dt_size = mybir.dt.size(in_ap.dtype)
    stride_bytes = elem_step * dt_size
    assert stride_bytes % 256 == 0 and stride_bytes // 256 < 256
    assert in_ap.ap[0][0] == elem_step and in_ap.ap[-1][1] == elem_size
    assert out_ap.ap[-1][1] == elem_size
    assert out_ap.ap[0][1] * out_ap.ap[1][1] == cdiv(num_idxs, 128) * 128
    _in_ap = gp.lower_ap_dma(in_ap, for_custom_bir_dma=True)
    _idxs_ap = gp.lower_ap(idxs_ap)
    _out_ap = gp.lower_ap(out_ap)
    return gp.add_instruction(
        mybir.InstDMAGatherAnt(
            name=gp.bass.get_next_instruction_name(),
            ins=[*_in_ap, _idxs_ap, gp.lower_val_access(gp.to_reg(num_idxs))],
            outs=[_out_ap],
            transpose=False,
            num_idxs=num_idxs,
            elem_size=elem_size,
            stride_bytes_256=stride_bytes // 256,
            gen_mode=0,
            single_packet=False,
            queue_num=queue_num,
            sbuf_tokens_per_rank=0,
            sbuf_free_dim_per_rank=0,
            sbuf_free_dim_pad_per_rank=0,
            sbuf_byte_offset=0,
        ))


def _build_program(sched, repeat=1):
    import concourse.bass as bass
    import concourse.bacc as bacc
    import concourse.mybir as mybir
    import concourse.tile as tile

    T_run = sched["T_run"]
    run_tile_off = sched["run_tile_off"]
    tot_tiles = sched["tot_tiles"]
    tot_slots = sched["tot_slots"]
    call_tiles = sched["call_tiles"]
    bw_tiles = sched["bw_tiles"]
    chunk_base = sched["chunk_base"]
    max_bw_tiles = int(bw_tiles.max())

    nc = bacc.Bacc(None, target_bir_lowering=False, debug=False,
                   num_swdge_queues=4)
    x_in = nc.declare_dram_parameter("x", [N_NODES, XPAD], mybir.dt.bfloat16,
                                     isOutput=False)
    idx_in = nc.declare_dram_parameter("idx", [P, tot_slots // 16],
                                       mybir.dt.int16, isOutput=False)
    dstl_in = nc.declare_dram_parameter("dstl", [P, tot_tiles],
                                        mybir.dt.float32, isOutput=False)
    iota_in = nc.declare_dram_parameter("iota", [P, SW], mybir.dt.bfloat16,
                                        isOutput=False)
    y_out = nc.declare_dram_parameter("y", [P, N_GRP * D_FEAT],
                                      mybir.dt.float32, isOutput=True)

    with tile.TileContext(nc) as tc:
        with (
            tc.tile_pool(name="const", bufs=1) as constp,
            tc.tile_pool(name="idxp", bufs=1) as idxp,
            tc.tile_pool(name="dstlp", bufs=1) as dstlp,
            tc.tile_pool(name="outp", bufs=1) as outp,
            tc.tile_pool(name="msgp", bufs=6) as msgp,
            tc.tile_pool(name="ohp", bufs=8) as ohp,
            tc.tile_pool(name="psp", bufs=4, space="PSUM") as psp,
        ):
            iota_sb = constp.tile([P, SW], mybir.dt.bfloat16)
            nc.sync.dma_start(out=iota_sb[:], in_=iota_in[:, :])
            zlhs = constp.tile([P, P], mybir.dt.bfloat16)
            nc.gpsimd.memset(zlhs[:], 0.0)
            zrhs = constp.tile([P, GRP_PER_BW * D_FEAT], mybir.dt.bfloat16)
            nc.gpsimd.memset(zrhs[:], 0.0)
            idx_sb = idxp.tile([P, tot_slots // 16], mybir.dt.int16)
            nc.sync.dma_start(out=idx_sb[:], in_=idx_in[:, :])
            dstl_sb = dstlp.tile([P, tot_tiles], mybir.dt.float32)
            nc.sync.dma_start(out=dstl_sb[:], in_=dstl_in[:, :])
            out_sb = outp.tile([P, N_GRP * D_FEAT], mybir.dt.float32)

            qn = 0
            for _rep in range(repeat):
                for b in range(N_BW):
                    T_bw = int(bw_tiles[b])
                    bt0 = int(run_tile_off[b, 0, 0])
                    psum = psp.tile([P, GRP_PER_BW * D_FEAT],
                                    mybir.dt.float32, space="PSUM")
                    nc.tensor.matmul(out=psum[:], lhsT=zlhs[:], rhs=zrhs[:],
                                     start=True, stop=False)
                    msg = msgp.tile([P, max_bw_tiles, D_FEAT],
                                    mybir.dt.bfloat16, tag="msg")
                    for c in range(N_CK):
                        Tc = int(call_tiles[b, c])
                        if Tc == 0:
                            continue
                        ct0 = int(run_tile_off[b, c, 0])
                        s0 = ct0 * P
                        _dma_gather_small(
                            nc.gpsimd, mybir,
                            out_ap=msg[:, ct0 - bt0:ct0 - bt0 + Tc, :],
                            in_ap=x_in[int(chunk_base[c]):, 0:D_FEAT],
                            idxs_ap=idx_sb[:, s0 // 16:(s0 + Tc * P) // 16],
                            num_idxs=Tc * P,
                            elem_size=D_FEAT,
                            elem_step=XPAD,
                            queue_num=qn % 4,
                        )
                        qn += 1
                    done = 0
                    for c in range(N_CK):
                        for gi in range(GRP_PER_BW):
                            T = int(T_run[b, c, gi])
                            for tl in range(T):
                                gt = int(run_tile_off[b, c, gi]) + tl
                                oh = ohp.tile([P, SW], mybir.dt.bfloat16,
                                              tag="oh")
                                nc.vector.tensor_scalar(
                                    out=oh[:],
                                    in0=iota_sb[:],
                                    scalar1=dstl_sb[:, gt:gt + 1],
                                    scalar2=None,
                                    op0=mybir.AluOpType.is_equal,
                                )
                                done += 1
                                nc.tensor.matmul(
                                    out=psum[:, gi * D_FEAT:(gi + 1) * D_FEAT],
                                    lhsT=oh[:],
                                    rhs=msg[:, gt - bt0, :],
                                    start=False,
                                    stop=(done == T_bw),
                                )
                    nc.scalar.copy(
                        out=out_sb[:, b * GRP_PER_BW * D_FEAT:
                                   (b + 1) * GRP_PER_BW * D_FEAT],
                        in_=psum[:],
                    )
            nc.sync.dma_start(out=y_out[:, :], in_=out_sb[:])
    nc.compile()
    return nc


def build(x, edge_index, repeat=1):
    import ml_dtypes
    x = np.asarray(x, dtype=np.float32)
    edge_index = np.asarray(edge_index)
    assert x.shape == (N_NODES, D_FEAT), x.shape

    sched, idx_cores, dstl_cores, meta = _preprocess(edge_index)
    nc = _build_program(sched, repeat=repeat)

    x_pad = np.zeros((N_NODES, XPAD), np.float32)
    x_pad[:, :D_FEAT] = x[meta["new_order"]]
    x_pad = x_pad.astype(ml_dtypes.bfloat16)
    iota = np.tile(np.arange(SW, dtype=np.float32), (P, 1)).astype(
        ml_dtypes.bfloat16)
    in_maps = []
    for cr in range(N_CORES):
        in_maps.append({
            "x": x_pad,
            "idx": idx_cores[cr],
            "dstl": dstl_cores[cr],
            "iota": iota,
        })
    return nc, in_maps, meta


def postprocess(results, meta, n_nodes):
    core_id = meta["core_id"]
    grp_id = meta["grp_id"]
    pos_id = meta["pos_id"]
    out = np.empty((n_nodes, D_FEAT), np.float32)
    for cr in range(N_CORES):
        y = np.asarray(results[cr]["y"], dtype=np.float32).reshape(
            P, N_GRP, D_FEAT)
        m = core_id == cr
        out[m] = y[pos_id[m], grp_id[m], :]
    return out


def kernel(x, edge_index):
    n_nodes = np.asarray(x).shape[0]
    nc, in_maps, meta = build(x, edge_index)
    from concourse.bass_utils import run_bass_kernel_spmd
    res = run_bass_kernel_spmd(nc, in_maps, list(range(N_CORES)))
    return postprocess(res.results, meta, n_nodes)


if __name__ == "__main__":
    import reference
    inputs = reference.setup_inputs()
    inputs = {k: np.asarray(v) for k, v in inputs.items()}
    got = kernel(**inputs)
    want = np.asarray(reference.reference(**inputs))
    denom = max(np.abs(want).max(), 1e-30)
    rel = np.abs(got - want).max() / denom
    print(f"Relative error: {rel:.3e}")
